# revision 50
# baseline (speedup 1.0000x reference)
"""Trainium2 Bass kernel for CAGKE (Gaussian-kernel spike embedding), v10.

Math: psedu[t] = sum_d softmax(weight)[d] * (spikes (*) K_d)[t] + noise[t],
then global min-max normalization. The softmax-weighted kernel-bank sum
commutes with the convolution (linearity), so psedu = spikes (*) kbar +
noise with kbar(delta) = sum_d sw_d*(C/sigma_d)*exp(-delta^2/(2 s_d^2)),
live taps |delta| <= 44 in f32.

Host-side input prep (O(T) / O(D*taps), ~0.1% of module FLOPs): the
D=128 kernel bank is folded into the banded-Toeplitz operand m_all; the
spike train is thresholded and laid out as the shifted transposed grid
spT[qt, c'] = (X[128c'-45+qt] > 0.5); the softmax denominator (min-max
norm is invariant under positive affine maps) is folded into the noise
as noise2 = (sum_d exp(w_d)/C) * noise. All constants (broadcast ones
rows, transpose identity) ride the input DMAs - the device program has
zero preamble engine ops.

The device runs the convolution - 99.9% of the work - as two
128-contraction PE matmuls over the shifted spike grid:

  psedu[128c+p] = sum_qt spT[qt, c]  * m_all[qt, 128+p]
               +  sum_qt spT[qt, c+1]* m_all[qt, p]

DMA placement (profiled): one big operand per HWDGE ring (m_all on
sync, spT on scalar, each a single DIRECT2D so the conv operands land
earliest); noise2/id64/cst2 ride the gpsimd SWDGE ring in need order.
Tail: DVE add + row max / -min, PE transpose of the [64,2] row-stat
tile, one column max into the pre-zeroed cst2 cells, two ones-row
broadcast matmuls ([range; -gmin] on all 64 partitions), reciprocal,
and a single fused (x + (-gmin)) * inv normalize; stores run as row
halves with the two DIRECT2Ds issued in parallel on both HWDGE rings.

Measured-window notes (neuron-profile): the window opens at the
framework's const-AP memsets (~5.9us, unavoidable), so only the
absolute finish time matters; after the last store there is a fixed
~10.4us runtime epilogue (store ring latency + tile-scope close + an
~7.4us multi-core stop handshake) that no program change moves.
Rejected via profiling: tensor_tensor_reduce op1=max (runtime INTERNAL
error), DVE stream transpose (~200ns/block > PE round trip), PSUM-bank
warmup matmuls (no effect), ACT-seeded PSUM noise add (gates the conv
on the SWDGE noise arrival).

All 8 cores run the identical replicated program (total I/O ~160KB;
collectives would cost more than they save); host takes core 0's output.
"""

import os
import sys

for _p in ("/opt/trn_rl_repo", "/root/.axon_site/_ro/trn_rl_repo"):
    if os.path.isdir(_p) and _p not in sys.path:
        sys.path.insert(0, _p)

import numpy as np

T = 8192  # in_length
D = 128  # embed_dim (kernel bank size)
NCORES = 8
COLS = T // 128  # 64 output blocks of 128 time steps
SCOLS = COLS + 1  # 65 shifted spike blocks
SHIFT = 45  # spike grid shift: block c' covers X[128c'-45 : 128c'+83)
GAUSS_C = 0.39894228  # 1/sqrt(2*pi) as hardcoded in the source module

_CACHE = {}


def _build_bass():
    import concourse.bass as bass
    import concourse.tile as tile
    from concourse import bacc, mybir

    f32 = mybir.dt.float32
    bf16 = mybir.dt.bfloat16
    nc = bacc.Bacc("TRN2", target_bir_lowering=False, debug=False, num_devices=NCORES)

    # one combined operand tile (host-packed, bf16): cols 0:128 = m1
    # (mm2 rhs), 128:256 = m0 (mm1 rhs), 256:321 = spT, 322:386 = ones
    # rows, 386:387 = runtime g cell
    CC = 388
    c_d = nc.dram_tensor("comb", [128, CC], bf16, kind="ExternalInput")
    n_d = nc.dram_tensor("noise2", [COLS, 128], f32, kind="ExternalInput")
    i_d = nc.dram_tensor("id64", [COLS, COLS], bf16, kind="ExternalInput")
    o_d = nc.dram_tensor("out", [COLS, 128], f32, kind="ExternalOutput")

    with tile.TileContext(nc) as tc:
        with (
            tc.tile_pool(name="sb", bufs=1) as sb,
            tc.tile_pool(name="ps", bufs=1, space="PSUM") as ps,
        ):
            # ---- input DMAs: spike grid halves split across both HWDGE
            # rings, kernel table on sync, noise on scalar ----
            # ---- input DMAs. Everything bf16 rides one combined tile,
            # row-split across both HWDGE rings (two ~64-row DIRECT2Ds
            # issue in parallel; 192 descriptor-rows total instead of
            # 322), so the conv operands land ~1us earlier. noise2 (f32)
            # follows on sync; the SWDGE ring is unused. ----
            comb = sb.tile([128, CC], bf16, tag="L")
            nrm = sb.tile([COLS, 128], f32)
            id64 = sb.tile([COLS, COLS], bf16)
            nc.sync.dma_start(
                out=comb[0:64, :],
                in_=bass.AP(tensor=c_d.ap().tensor, offset=0, ap=[[CC, 64], [1, CC]]),
            )
            nc.scalar.dma_start(
                out=comb[64:128, :],
                in_=bass.AP(
                    tensor=c_d.ap().tensor, offset=64 * CC, ap=[[CC, 64], [1, CC]]
                ),
            )
            nc.sync.dma_start(out=nrm[:], in_=n_d.ap())
            nc.scalar.dma_start(out=id64[:], in_=i_d.ap())
            # ---- banded conv: psedu_rm[c, p] = sum_qt spT[qt, c+b] M_b[qt, p] ----
            conv_ps = ps.tile([COLS, 128], f32, tag="ps_c")
            stat_ps = ps.tile([COLS, 2], f32, tag="ps_s")
            nc.tensor.matmul(
                conv_ps[:], lhsT=comb[:, 256:320], rhs=comb[:, 128:256],
                start=True, stop=False,
            )
            nc.tensor.matmul(
                conv_ps[:], lhsT=comb[:, 257:321], rhs=comb[:, 0:128],
                start=False, stop=True,
            )

            # ---- fused noise-add + row-max, then row-min ----
            # ---- noise add (bf16 result: 2x DVE rate on the reduces;
            # ~3e-3 rel err, 7x under the gate), then per-row max/-min ----
            ps_rm = sb.tile([COLS, 128], bf16)
            pk = sb.tile([COLS, 2], bf16)  # col0 = -row min, col1 = row max
            nc.vector.tensor_tensor(
                out=ps_rm[:], in0=conv_ps[:], in1=nrm[:], op=mybir.AluOpType.add,
            )
            nc.vector.tensor_reduce(
                out=pk[:, 1:2], in_=ps_rm[:], axis=mybir.AxisListType.X,
                op=mybir.AluOpType.max,
            )
            nc.vector.tensor_reduce(
                out=pk[:, 0:1], in_=ps_rm[:], axis=mybir.AxisListType.X,
                op=mybir.AluOpType.min, negate=True,
            )
            # ---- global min/max: PE transpose of pk (bf16 identity from
            # the combined tile), one column reduce into the g cell, then
            # two ones-row broadcast matmuls ----
            pk_ps = ps.tile([2, COLS], bf16, tag="ps_k")
            nc.tensor.transpose(pk_ps[:], pk[:], id64[:])
            nc.vector.tensor_reduce(
                out=comb[0:2, 386:387], in_=pk_ps[:], axis=mybir.AxisListType.X,
                op=mybir.AluOpType.max,
            )  # [-gmin, gmax]
            # two ones-row broadcast matmuls off the same g cell:
            # contraction over both rows -> range; over row 0 -> -gmin
            nc.tensor.matmul(
                stat_ps[:, 0:1], lhsT=comb[0:2, 322:386], rhs=comb[0:2, 386:387],
                start=True, stop=True,
            )
            nc.tensor.matmul(
                stat_ps[:, 1:2], lhsT=comb[0:1, 322:386], rhs=comb[0:1, 386:387],
                start=True, stop=True,
            )  # stat[:, 0] = range, stat[:, 1] = -gmin
            inv_rng = sb.tile([COLS, 1], f32)
            nc.vector.reciprocal(inv_rng[:], stat_ps[:, 0:1])
            # single fused normalize (x + (-gmin)) * 1/range, then store
            # halves on both HWDGE rings so the issues run in parallel
            outt = sb.tile([COLS, 128], f32)
            nc.vector.tensor_scalar(
                out=outt[:], in0=ps_rm[:], scalar1=stat_ps[:, 1:2],
                scalar2=inv_rng[:, 0:1], op0=mybir.AluOpType.add,
                op1=mybir.AluOpType.mult,
            )
            nc.sync.dma_start(out=o_d.ap()[0:32, :], in_=outt[0:32, :])
            nc.scalar.dma_start(out=o_d.ap()[32:COLS, :], in_=outt[32:COLS, :])

    nc.compile()
    return nc


def _get_nc():
    if "nc" not in _CACHE:
        _CACHE["nc"] = _build_bass()
    return _CACHE["nc"]


def _run(in_map, trace=False, **kwargs):
    from concourse.bass_utils import run_bass_kernel_spmd

    nc = _get_nc()
    return run_bass_kernel_spmd(
        nc, [in_map] * NCORES, core_ids=list(range(NCORES)), trace=trace, **kwargs
    )


def _prepare(X, weight, noise, sigma):
    """Host-side input prep: fold the kernel bank into the banded-Toeplitz
    conv operand (linearity of the softmax-weighted sum; min-max norm is
    invariant to the overall softmax scale, which lands on the noise),
    threshold + shift-transpose the spike train."""
    import ml_dtypes

    X = np.ascontiguousarray(X, dtype=np.float32).reshape(T)
    weight = np.asarray(weight, dtype=np.float64).reshape(D)
    noise = np.ascontiguousarray(noise, dtype=np.float32).reshape(COLS, 128)
    sigma = np.asarray(sigma, dtype=np.float64).reshape(D)

    # kbar'(delta) = sum_d exp(w_d)/sigma_d * exp(-delta^2/(2 sigma_d^2));
    # psedu' = kbar' (*) spikes + (esum/C)*noise is a positive affine image
    # of the reference psedu, so the min-max normalized output matches.
    delta = np.arange(-256, 256, dtype=np.float64)
    kb = (
        (np.exp(weight) / sigma)[:, None]
        * np.exp(-(delta[None, :] ** 2) / (2.0 * sigma * sigma)[:, None])
    ).sum(0)  # kb[j] = kbar'(j - 256)

    # m_all[qt, p + 128*(1-b)] = kbar'(p + 44 - qt - 128b), b in {0, 1}
    qt = np.arange(128)[:, None]
    p = np.arange(128)[None, :]
    m0 = kb[256 + p + 44 - qt]  # b=0 -> comb cols 128:256
    m1 = kb[256 + p + 44 - qt - 128]  # b=1 -> comb cols 0:128

    # spT[qt, c'] = spikes[128c' - 45 + qt], zero outside [0, T)
    spikes = (X > 0.5).astype(np.float32)
    c = np.arange(SCOLS)[None, :]
    idx = 128 * c - SHIFT + np.arange(128)[:, None]
    valid = (idx >= 0) & (idx < T)
    spT = np.where(valid, spikes[np.clip(idx, 0, T - 1)], 0.0)

    es = np.exp(weight).sum() / GAUSS_C
    noise2 = (es * noise.astype(np.float64)).astype(np.float32)

    # combined bf16 tile: [m1 | m0 | spT | pad | ones rows | g cell],
    # matching the column offsets hardcoded on-device
    comb = np.zeros((128, 388), dtype=np.float64)
    comb[:, 0:128] = m1
    comb[:, 128:256] = m0
    comb[:, 256 : 256 + SCOLS] = spT
    comb[0:2, 322:386] = 1.0  # ones rows for the broadcast matmuls

    return {
        "comb": comb.astype(ml_dtypes.bfloat16),
        "noise2": noise2,
        "id64": np.eye(COLS, dtype=np.float64).astype(ml_dtypes.bfloat16),
    }


def kernel(X, weight, noise, sigma):
    in_map = _prepare(X, weight, noise, sigma)
    try:
        res = _run(in_map).results
    except Exception:
        # transient runtime INTERNAL errors (device wedge) clear on retry
        res = _run(in_map).results
    return res[0]["out"].reshape(1, T)


# revision 51
# speedup vs baseline: 1.0081x; 1.0081x over previous
"""Trainium2 Bass kernel for CAGKE (Gaussian-kernel spike embedding), v10.

Math: psedu[t] = sum_d softmax(weight)[d] * (spikes (*) K_d)[t] + noise[t],
then global min-max normalization. The softmax-weighted kernel-bank sum
commutes with the convolution (linearity), so psedu = spikes (*) kbar +
noise with kbar(delta) = sum_d sw_d*(C/sigma_d)*exp(-delta^2/(2 s_d^2)),
live taps |delta| <= 44 in f32.

Host-side input prep (O(T) / O(D*taps), ~0.1% of module FLOPs): the
D=128 kernel bank is folded into the banded-Toeplitz operand m_all; the
spike train is thresholded and laid out as the shifted transposed grid
spT[qt, c'] = (X[128c'-45+qt] > 0.5); the softmax denominator (min-max
norm is invariant under positive affine maps) is folded into the noise
as noise2 = (sum_d exp(w_d)/C) * noise. All constants (broadcast ones
rows, transpose identity) ride the input DMAs - the device program has
zero preamble engine ops.

The device runs the convolution - 99.9% of the work - as two
128-contraction PE matmuls over the shifted spike grid:

  psedu[128c+p] = sum_qt spT[qt, c]  * m_all[qt, 128+p]
               +  sum_qt spT[qt, c+1]* m_all[qt, p]

DMA placement (profiled): one big operand per HWDGE ring (m_all on
sync, spT on scalar, each a single DIRECT2D so the conv operands land
earliest); noise2/id64/cst2 ride the gpsimd SWDGE ring in need order.
Tail: DVE add + row max / -min, PE transpose of the [64,2] row-stat
tile, one column max into the pre-zeroed cst2 cells, two ones-row
broadcast matmuls ([range; -gmin] on all 64 partitions), reciprocal,
and a single fused (x + (-gmin)) * inv normalize; stores run as row
halves with the two DIRECT2Ds issued in parallel on both HWDGE rings.

Measured-window notes (neuron-profile): the window opens at the
framework's const-AP memsets (~5.9us, unavoidable), so only the
absolute finish time matters; after the last store there is a fixed
~10.4us runtime epilogue (store ring latency + tile-scope close + an
~7.4us multi-core stop handshake) that no program change moves.
Rejected via profiling: tensor_tensor_reduce op1=max (runtime INTERNAL
error), DVE stream transpose (~200ns/block > PE round trip), PSUM-bank
warmup matmuls (no effect), ACT-seeded PSUM noise add (gates the conv
on the SWDGE noise arrival).

All 8 cores run the identical replicated program (total I/O ~160KB;
collectives would cost more than they save); host takes core 0's output.
"""

import os
import sys

for _p in ("/opt/trn_rl_repo", "/root/.axon_site/_ro/trn_rl_repo"):
    if os.path.isdir(_p) and _p not in sys.path:
        sys.path.insert(0, _p)

import numpy as np

T = 8192  # in_length
D = 128  # embed_dim (kernel bank size)
NCORES = 8
COLS = T // 128  # 64 output blocks of 128 time steps
SCOLS = COLS + 1  # 65 shifted spike blocks
SHIFT = 45  # spike grid shift: block c' covers X[128c'-45 : 128c'+83)
GAUSS_C = 0.39894228  # 1/sqrt(2*pi) as hardcoded in the source module

_CACHE = {}


def _build_bass():
    import concourse.bass as bass
    import concourse.tile as tile
    from concourse import bacc, mybir

    f32 = mybir.dt.float32
    bf16 = mybir.dt.bfloat16
    nc = bacc.Bacc("TRN2", target_bir_lowering=False, debug=False, num_devices=NCORES)

    # one combined operand tile (host-packed, bf16): cols 0:128 = m1
    # (mm2 rhs), 128:256 = m0 (mm1 rhs), 256:321 = spT, 322:386 = ones
    # rows, 386:387 = runtime g cell
    CC = 388
    c_d = nc.dram_tensor("comb", [128, CC], bf16, kind="ExternalInput")
    n_d = nc.dram_tensor("noise2", [COLS, 128], f32, kind="ExternalInput")
    i_d = nc.dram_tensor("id64", [COLS, COLS], bf16, kind="ExternalInput")
    o_d = nc.dram_tensor("out", [COLS, 128], f32, kind="ExternalOutput")

    with tile.TileContext(nc) as tc:
        with (
            tc.tile_pool(name="sb", bufs=1) as sb,
            tc.tile_pool(name="ps", bufs=1, space="PSUM") as ps,
        ):
            # ---- input DMAs: spike grid halves split across both HWDGE
            # rings, kernel table on sync, noise on scalar ----
            # ---- input DMAs. Everything bf16 rides one combined tile,
            # row-split across both HWDGE rings (two ~64-row DIRECT2Ds
            # issue in parallel; 192 descriptor-rows total instead of
            # 322), so the conv operands land ~1us earlier. noise2 (f32)
            # follows on sync; the SWDGE ring is unused. ----
            comb = sb.tile([128, CC], bf16, tag="L")
            nrm = sb.tile([COLS, 128], f32)
            id64 = sb.tile([COLS, COLS], bf16)
            nc.sync.dma_start(
                out=comb[0:64, :],
                in_=bass.AP(tensor=c_d.ap().tensor, offset=0, ap=[[CC, 64], [1, CC]]),
            )
            nc.scalar.dma_start(
                out=comb[64:128, :],
                in_=bass.AP(
                    tensor=c_d.ap().tensor, offset=64 * CC, ap=[[CC, 64], [1, CC]]
                ),
            )
            nc.sync.dma_start(out=nrm[:], in_=n_d.ap())
            nc.scalar.dma_start(out=id64[:], in_=i_d.ap())
            # ---- banded conv: psedu_rm[c, p] = sum_qt spT[qt, c+b] M_b[qt, p] ----
            conv_ps = ps.tile([COLS, 128], f32, tag="ps_c")
            stat_ps = ps.tile([COLS, 2], f32, tag="ps_s")
            nc.tensor.matmul(
                conv_ps[:], lhsT=comb[:, 256:320], rhs=comb[:, 128:256],
                start=True, stop=False,
            )
            nc.tensor.matmul(
                conv_ps[:], lhsT=comb[:, 257:321], rhs=comb[:, 0:128],
                start=False, stop=True,
            )

            # ---- fused noise-add + row-max, then row-min ----
            # ---- noise add, then per-row max / -min (bf16 ps_rm gives no
            # DVE speedup on TRN2 - the 16-bit 2x path doesn't engage - so
            # keep f32 for accuracy; pk is bf16 for the bf16 transpose) ----
            ps_rm = sb.tile([COLS, 128], f32)
            pk = sb.tile([COLS, 2], bf16)  # col0 = -row min, col1 = row max
            nc.vector.tensor_tensor(
                out=ps_rm[:], in0=conv_ps[:], in1=nrm[:], op=mybir.AluOpType.add,
            )
            nc.vector.tensor_reduce(
                out=pk[:, 1:2], in_=ps_rm[:], axis=mybir.AxisListType.X,
                op=mybir.AluOpType.max,
            )
            nc.vector.tensor_reduce(
                out=pk[:, 0:1], in_=ps_rm[:], axis=mybir.AxisListType.X,
                op=mybir.AluOpType.min, negate=True,
            )
            # ---- global min/max: PE transpose of pk (bf16 identity from
            # the combined tile), one column reduce into the g cell, then
            # two ones-row broadcast matmuls ----
            pk_ps = ps.tile([2, COLS], bf16, tag="ps_k")
            nc.tensor.transpose(pk_ps[:], pk[:], id64[:])
            nc.vector.tensor_reduce(
                out=comb[0:2, 386:387], in_=pk_ps[:], axis=mybir.AxisListType.X,
                op=mybir.AluOpType.max,
            )  # [-gmin, gmax]
            # two ones-row broadcast matmuls off the same g cell:
            # contraction over both rows -> range; over row 0 -> -gmin
            nc.tensor.matmul(
                stat_ps[:, 0:1], lhsT=comb[0:2, 322:386], rhs=comb[0:2, 386:387],
                start=True, stop=True,
            )
            nc.tensor.matmul(
                stat_ps[:, 1:2], lhsT=comb[0:1, 322:386], rhs=comb[0:1, 386:387],
                start=True, stop=True,
            )  # stat[:, 0] = range, stat[:, 1] = -gmin
            inv_rng = sb.tile([COLS, 1], f32)
            nc.vector.reciprocal(inv_rng[:], stat_ps[:, 0:1])
            # single fused normalize (x + (-gmin)) * 1/range, then store
            # halves on both HWDGE rings so the issues run in parallel
            outt = sb.tile([COLS, 128], f32)
            nc.vector.tensor_scalar(
                out=outt[:], in0=ps_rm[:], scalar1=stat_ps[:, 1:2],
                scalar2=inv_rng[:, 0:1], op0=mybir.AluOpType.add,
                op1=mybir.AluOpType.mult,
            )
            nc.sync.dma_start(out=o_d.ap()[0:32, :], in_=outt[0:32, :])
            nc.scalar.dma_start(out=o_d.ap()[32:COLS, :], in_=outt[32:COLS, :])

    nc.compile()
    return nc


def _get_nc():
    if "nc" not in _CACHE:
        _CACHE["nc"] = _build_bass()
    return _CACHE["nc"]


def _run(in_map, trace=False, **kwargs):
    from concourse.bass_utils import run_bass_kernel_spmd

    nc = _get_nc()
    return run_bass_kernel_spmd(
        nc, [in_map] * NCORES, core_ids=list(range(NCORES)), trace=trace, **kwargs
    )


def _prepare(X, weight, noise, sigma):
    """Host-side input prep: fold the kernel bank into the banded-Toeplitz
    conv operand (linearity of the softmax-weighted sum; min-max norm is
    invariant to the overall softmax scale, which lands on the noise),
    threshold + shift-transpose the spike train."""
    import ml_dtypes

    X = np.ascontiguousarray(X, dtype=np.float32).reshape(T)
    weight = np.asarray(weight, dtype=np.float64).reshape(D)
    noise = np.ascontiguousarray(noise, dtype=np.float32).reshape(COLS, 128)
    sigma = np.asarray(sigma, dtype=np.float64).reshape(D)

    # kbar'(delta) = sum_d exp(w_d)/sigma_d * exp(-delta^2/(2 sigma_d^2));
    # psedu' = kbar' (*) spikes + (esum/C)*noise is a positive affine image
    # of the reference psedu, so the min-max normalized output matches.
    delta = np.arange(-256, 256, dtype=np.float64)
    kb = (
        (np.exp(weight) / sigma)[:, None]
        * np.exp(-(delta[None, :] ** 2) / (2.0 * sigma * sigma)[:, None])
    ).sum(0)  # kb[j] = kbar'(j - 256)

    # m_all[qt, p + 128*(1-b)] = kbar'(p + 44 - qt - 128b), b in {0, 1}
    qt = np.arange(128)[:, None]
    p = np.arange(128)[None, :]
    m0 = kb[256 + p + 44 - qt]  # b=0 -> comb cols 128:256
    m1 = kb[256 + p + 44 - qt - 128]  # b=1 -> comb cols 0:128

    # spT[qt, c'] = spikes[128c' - 45 + qt], zero outside [0, T)
    spikes = (X > 0.5).astype(np.float32)
    c = np.arange(SCOLS)[None, :]
    idx = 128 * c - SHIFT + np.arange(128)[:, None]
    valid = (idx >= 0) & (idx < T)
    spT = np.where(valid, spikes[np.clip(idx, 0, T - 1)], 0.0)

    es = np.exp(weight).sum() / GAUSS_C
    noise2 = (es * noise.astype(np.float64)).astype(np.float32)

    # combined bf16 tile: [m1 | m0 | spT | pad | ones rows | g cell],
    # matching the column offsets hardcoded on-device
    comb = np.zeros((128, 388), dtype=np.float64)
    comb[:, 0:128] = m1
    comb[:, 128:256] = m0
    comb[:, 256 : 256 + SCOLS] = spT
    comb[0:2, 322:386] = 1.0  # ones rows for the broadcast matmuls

    return {
        "comb": comb.astype(ml_dtypes.bfloat16),
        "noise2": noise2,
        "id64": np.eye(COLS, dtype=np.float64).astype(ml_dtypes.bfloat16),
    }


def kernel(X, weight, noise, sigma):
    in_map = _prepare(X, weight, noise, sigma)
    try:
        res = _run(in_map).results
    except Exception:
        # transient runtime INTERNAL errors (device wedge) clear on retry
        res = _run(in_map).results
    return res[0]["out"].reshape(1, T)


# revision 52
# speedup vs baseline: 1.0128x; 1.0047x over previous
"""Trainium2 Bass kernel for CAGKE (Gaussian-kernel spike embedding), v10.

Math: psedu[t] = sum_d softmax(weight)[d] * (spikes (*) K_d)[t] + noise[t],
then global min-max normalization. The softmax-weighted kernel-bank sum
commutes with the convolution (linearity), so psedu = spikes (*) kbar +
noise with kbar(delta) = sum_d sw_d*(C/sigma_d)*exp(-delta^2/(2 s_d^2)),
live taps |delta| <= 44 in f32.

Host-side input prep (O(T) / O(D*taps), ~0.1% of module FLOPs): the
D=128 kernel bank is folded into the banded-Toeplitz operand m_all; the
spike train is thresholded and laid out as the shifted transposed grid
spT[qt, c'] = (X[128c'-45+qt] > 0.5); the softmax denominator (min-max
norm is invariant under positive affine maps) is folded into the noise
as noise2 = (sum_d exp(w_d)/C) * noise. All constants (broadcast ones
rows, transpose identity) ride the input DMAs - the device program has
zero preamble engine ops.

The device runs the convolution - 99.9% of the work - as two
128-contraction PE matmuls over the shifted spike grid:

  psedu[128c+p] = sum_qt spT[qt, c]  * m_all[qt, 128+p]
               +  sum_qt spT[qt, c+1]* m_all[qt, p]

DMA placement (profiled; the input pipe is descriptor-bound at ~4-6ns
per SBUF row): all bf16 operands - m1|m0|spT|ones|g-cell - are
host-packed into ONE [128, 388] tile, loaded as two 64-row halves with
one DIRECT2D per HWDGE ring issuing in parallel, so the conv operands
land ~1us earlier than separate loads; noise2 (f32) follows on sync and
the bf16 identity on scalar. The SWDGE ring is unused.
Tail: DVE add + row max / -min (f32: the DVE 16-bit 2x path does not
engage on TRN2, so bf16 here only costs accuracy), bf16 PE transpose of
the [64,2] row-stat tile, one column max into the g cell, two ones-row
broadcast matmuls ([range; -gmin] on all 64 partitions), reciprocal,
and a single fused (x + (-gmin)) * inv normalize; stores run as row
halves with the two DIRECT2Ds issued in parallel on both HWDGE rings.

Measured-window notes (neuron-profile): the window opens at the
framework's const-AP memsets (~5.9us, unavoidable), so only the
absolute finish time matters; after the last store there is a fixed
~10.4us runtime epilogue (store ring latency + tile-scope close + an
~7.4us multi-core stop handshake) that no program change moves.
Rejected via profiling: tensor_tensor_reduce op1=max (runtime INTERNAL
error), DVE stream transpose (~200ns/block > PE round trip), PSUM-bank
warmup matmuls (no effect), ACT-seeded PSUM noise add (gates the conv
on the SWDGE noise arrival).

All 8 cores run the identical replicated program (total I/O ~160KB;
collectives would cost more than they save); host takes core 0's output.
"""

import os
import sys

for _p in ("/opt/trn_rl_repo", "/root/.axon_site/_ro/trn_rl_repo"):
    if os.path.isdir(_p) and _p not in sys.path:
        sys.path.insert(0, _p)

import numpy as np

T = 8192  # in_length
D = 128  # embed_dim (kernel bank size)
NCORES = 8
COLS = T // 128  # 64 output blocks of 128 time steps
SCOLS = COLS + 1  # 65 shifted spike blocks
SHIFT = 45  # spike grid shift: block c' covers X[128c'-45 : 128c'+83)
GAUSS_C = 0.39894228  # 1/sqrt(2*pi) as hardcoded in the source module

_CACHE = {}


def _build_bass():
    import concourse.bass as bass
    import concourse.tile as tile
    from concourse import bacc, mybir

    f32 = mybir.dt.float32
    bf16 = mybir.dt.bfloat16
    nc = bacc.Bacc("TRN2", target_bir_lowering=False, debug=False, num_devices=NCORES)

    # one combined operand tile (host-packed, bf16): cols 0:128 = m1
    # (mm2 rhs), 128:256 = m0 (mm1 rhs), 256:321 = spT, 322:386 = ones
    # rows, 386:387 = runtime g cell
    CC = 388
    c_d = nc.dram_tensor("comb", [128, CC], bf16, kind="ExternalInput")
    n_d = nc.dram_tensor("noise2", [COLS, 128], f32, kind="ExternalInput")
    i_d = nc.dram_tensor("id64", [COLS, COLS], bf16, kind="ExternalInput")
    o_d = nc.dram_tensor("out", [COLS, 128], f32, kind="ExternalOutput")

    with tile.TileContext(nc) as tc:
        with (
            tc.tile_pool(name="sb", bufs=1) as sb,
            tc.tile_pool(name="ps", bufs=1, space="PSUM") as ps,
        ):
            # ---- input DMAs: spike grid halves split across both HWDGE
            # rings, kernel table on sync, noise on scalar ----
            # ---- input DMAs. Everything bf16 rides one combined tile,
            # row-split across both HWDGE rings (two ~64-row DIRECT2Ds
            # issue in parallel; 192 descriptor-rows total instead of
            # 322), so the conv operands land ~1us earlier. noise2 (f32)
            # follows on sync; the SWDGE ring is unused. ----
            comb = sb.tile([128, CC], bf16, tag="L")
            nrm = sb.tile([COLS, 128], f32)
            id64 = sb.tile([COLS, COLS], bf16)
            nc.sync.dma_start(
                out=comb[0:64, :],
                in_=bass.AP(tensor=c_d.ap().tensor, offset=0, ap=[[CC, 64], [1, CC]]),
            )
            nc.scalar.dma_start(
                out=comb[64:128, :],
                in_=bass.AP(
                    tensor=c_d.ap().tensor, offset=64 * CC, ap=[[CC, 64], [1, CC]]
                ),
            )
            nc.sync.dma_start(out=nrm[:], in_=n_d.ap())
            nc.scalar.dma_start(out=id64[:], in_=i_d.ap())
            # ---- banded conv: psedu_rm[c, p] = sum_qt spT[qt, c+b] M_b[qt, p] ----
            conv_ps = ps.tile([COLS, 128], f32, tag="ps_c")
            stat_ps = ps.tile([COLS, 2], f32, tag="ps_s")
            nc.tensor.matmul(
                conv_ps[:], lhsT=comb[:, 256:320], rhs=comb[:, 128:256],
                start=True, stop=False,
            )
            nc.tensor.matmul(
                conv_ps[:], lhsT=comb[:, 257:321], rhs=comb[:, 0:128],
                start=False, stop=True,
            )

            # ---- fused noise-add + row-max, then row-min ----
            # ---- noise add, then per-row max / -min (bf16 ps_rm gives no
            # DVE speedup on TRN2 - the 16-bit 2x path doesn't engage - so
            # keep f32 for accuracy; pk is bf16 for the bf16 transpose) ----
            ps_rm = sb.tile([COLS, 128], f32)
            pk = sb.tile([COLS, 2], bf16)  # col0 = -row min, col1 = row max
            nc.vector.tensor_tensor(
                out=ps_rm[:], in0=conv_ps[:], in1=nrm[:], op=mybir.AluOpType.add,
            )
            nc.vector.tensor_reduce(
                out=pk[:, 1:2], in_=ps_rm[:], axis=mybir.AxisListType.X,
                op=mybir.AluOpType.max,
            )
            nc.vector.tensor_reduce(
                out=pk[:, 0:1], in_=ps_rm[:], axis=mybir.AxisListType.X,
                op=mybir.AluOpType.min, negate=True,
            )
            # ---- global min/max: PE transpose of pk (bf16 identity from
            # the combined tile), one column reduce into the g cell, then
            # two ones-row broadcast matmuls ----
            pk_ps = ps.tile([2, COLS], bf16, tag="ps_k")
            nc.tensor.transpose(pk_ps[:], pk[:], id64[:])
            nc.vector.tensor_reduce(
                out=comb[0:2, 386:387], in_=pk_ps[:], axis=mybir.AxisListType.X,
                op=mybir.AluOpType.max,
            )  # [-gmin, gmax]
            # two ones-row broadcast matmuls off the same g cell:
            # contraction over both rows -> range; over row 0 -> -gmin
            nc.tensor.matmul(
                stat_ps[:, 0:1], lhsT=comb[0:2, 322:386], rhs=comb[0:2, 386:387],
                start=True, stop=True,
            )
            nc.tensor.matmul(
                stat_ps[:, 1:2], lhsT=comb[0:1, 322:386], rhs=comb[0:1, 386:387],
                start=True, stop=True,
            )  # stat[:, 0] = range, stat[:, 1] = -gmin
            inv_rng = sb.tile([COLS, 1], f32)
            nc.vector.reciprocal(inv_rng[:], stat_ps[:, 0:1])
            # single fused normalize (x + (-gmin)) * 1/range, then store
            # halves on both HWDGE rings so the issues run in parallel
            outt = sb.tile([COLS, 128], f32)
            nc.vector.tensor_scalar(
                out=outt[:], in0=ps_rm[:], scalar1=stat_ps[:, 1:2],
                scalar2=inv_rng[:, 0:1], op0=mybir.AluOpType.add,
                op1=mybir.AluOpType.mult,
            )
            nc.sync.dma_start(out=o_d.ap()[0:32, :], in_=outt[0:32, :])
            nc.scalar.dma_start(out=o_d.ap()[32:COLS, :], in_=outt[32:COLS, :])

    nc.compile()
    return nc


def _get_nc():
    if "nc" not in _CACHE:
        _CACHE["nc"] = _build_bass()
    return _CACHE["nc"]


def _run(in_map, trace=False, **kwargs):
    from concourse.bass_utils import run_bass_kernel_spmd

    nc = _get_nc()
    return run_bass_kernel_spmd(
        nc, [in_map] * NCORES, core_ids=list(range(NCORES)), trace=trace, **kwargs
    )


def _prepare(X, weight, noise, sigma):
    """Host-side input prep: fold the kernel bank into the banded-Toeplitz
    conv operand (linearity of the softmax-weighted sum; min-max norm is
    invariant to the overall softmax scale, which lands on the noise),
    threshold + shift-transpose the spike train."""
    import ml_dtypes

    X = np.ascontiguousarray(X, dtype=np.float32).reshape(T)
    weight = np.asarray(weight, dtype=np.float64).reshape(D)
    noise = np.ascontiguousarray(noise, dtype=np.float32).reshape(COLS, 128)
    sigma = np.asarray(sigma, dtype=np.float64).reshape(D)

    # kbar'(delta) = sum_d exp(w_d)/sigma_d * exp(-delta^2/(2 sigma_d^2));
    # psedu' = kbar' (*) spikes + (esum/C)*noise is a positive affine image
    # of the reference psedu, so the min-max normalized output matches.
    delta = np.arange(-256, 256, dtype=np.float64)
    kb = (
        (np.exp(weight) / sigma)[:, None]
        * np.exp(-(delta[None, :] ** 2) / (2.0 * sigma * sigma)[:, None])
    ).sum(0)  # kb[j] = kbar'(j - 256)

    # m_all[qt, p + 128*(1-b)] = kbar'(p + 44 - qt - 128b), b in {0, 1}
    qt = np.arange(128)[:, None]
    p = np.arange(128)[None, :]
    m0 = kb[256 + p + 44 - qt]  # b=0 -> comb cols 128:256
    m1 = kb[256 + p + 44 - qt - 128]  # b=1 -> comb cols 0:128

    # spT[qt, c'] = spikes[128c' - 45 + qt], zero outside [0, T)
    spikes = (X > 0.5).astype(np.float32)
    c = np.arange(SCOLS)[None, :]
    idx = 128 * c - SHIFT + np.arange(128)[:, None]
    valid = (idx >= 0) & (idx < T)
    spT = np.where(valid, spikes[np.clip(idx, 0, T - 1)], 0.0)

    es = np.exp(weight).sum() / GAUSS_C
    noise2 = (es * noise.astype(np.float64)).astype(np.float32)

    # combined bf16 tile: [m1 | m0 | spT | pad | ones rows | g cell],
    # matching the column offsets hardcoded on-device
    comb = np.zeros((128, 388), dtype=np.float64)
    comb[:, 0:128] = m1
    comb[:, 128:256] = m0
    comb[:, 256 : 256 + SCOLS] = spT
    comb[0:2, 322:386] = 1.0  # ones rows for the broadcast matmuls

    return {
        "comb": comb.astype(ml_dtypes.bfloat16),
        "noise2": noise2,
        "id64": np.eye(COLS, dtype=np.float64).astype(ml_dtypes.bfloat16),
    }


def kernel(X, weight, noise, sigma):
    in_map = _prepare(X, weight, noise, sigma)
    try:
        res = _run(in_map).results
    except Exception:
        # transient runtime INTERNAL errors (device wedge) clear on retry
        res = _run(in_map).results
    return res[0]["out"].reshape(1, T)


# revision 54
# speedup vs baseline: 1.0411x; 1.0280x over previous
"""Trainium2 Bass kernel for CAGKE (Gaussian-kernel spike embedding), v10.

Math: psedu[t] = sum_d softmax(weight)[d] * (spikes (*) K_d)[t] + noise[t],
then global min-max normalization. The softmax-weighted kernel-bank sum
commutes with the convolution (linearity), so psedu = spikes (*) kbar +
noise with kbar(delta) = sum_d sw_d*(C/sigma_d)*exp(-delta^2/(2 s_d^2)),
live taps |delta| <= 44 in f32.

Host-side input prep (O(T) / O(D*taps), ~0.1% of module FLOPs): the
D=128 kernel bank is folded into the banded-Toeplitz operand m_all; the
spike train is thresholded and laid out as the shifted transposed grid
spT[qt, c'] = (X[128c'-45+qt] > 0.5); the softmax denominator (min-max
norm is invariant under positive affine maps) is folded into the noise
as noise2 = (sum_d exp(w_d)/C) * noise. All constants (broadcast ones
rows, transpose identity) ride the input DMAs - the device program has
zero preamble engine ops.

The device runs the convolution - 99.9% of the work - as two
128-contraction PE matmuls over the shifted spike grid:

  psedu[128c+p] = sum_qt spT[qt, c]  * m_all[qt, 128+p]
               +  sum_qt spT[qt, c+1]* m_all[qt, p]

DMA placement (profiled; the input pipe is descriptor-bound at ~4-6ns
per SBUF row): all bf16 operands - m1|m0|spT|ones|g-cell - are
host-packed into ONE [128, 388] tile, loaded as two 64-row halves with
one DIRECT2D per HWDGE ring issuing in parallel, so the conv operands
land ~1us earlier than separate loads; noise2 (f32) follows on sync and
the bf16 identity on scalar. The SWDGE ring is unused.
Tail: DVE add + row max / -min (f32: the DVE 16-bit 2x path does not
engage on TRN2, so bf16 here only costs accuracy), bf16 PE transpose of
the [64,2] row-stat tile, one column max into the g cell, two ones-row
broadcast matmuls ([range; -gmin] on all 64 partitions), reciprocal,
and a single fused (x + (-gmin)) * inv normalize; stores run as row
halves with the two DIRECT2Ds issued in parallel on both HWDGE rings.

Measured-window notes (neuron-profile): the window opens at the
framework's const-AP memsets (~5.9us, unavoidable), so only the
absolute finish time matters; after the last store there is a fixed
~10.4us runtime epilogue (store ring latency + tile-scope close + an
~7.4us multi-core stop handshake) that no program change moves.
Rejected via profiling: tensor_tensor_reduce op1=max (runtime INTERNAL
error), DVE stream transpose (~200ns/block > PE round trip), PSUM-bank
warmup matmuls (no effect), ACT-seeded PSUM noise add (gates the conv
on the SWDGE noise arrival).

All 8 cores run the identical replicated program (total I/O ~160KB;
collectives would cost more than they save); host takes core 0's output.
"""

import os
import sys

for _p in ("/opt/trn_rl_repo", "/root/.axon_site/_ro/trn_rl_repo"):
    if os.path.isdir(_p) and _p not in sys.path:
        sys.path.insert(0, _p)

import numpy as np

T = 8192  # in_length
D = 128  # embed_dim (kernel bank size)
NCORES = 8
COLS = T // 128  # 64 output blocks of 128 time steps
SCOLS = COLS + 1  # 65 shifted spike blocks
SHIFT = 45  # spike grid shift: block c' covers X[128c'-45 : 128c'+83)
GAUSS_C = 0.39894228  # 1/sqrt(2*pi) as hardcoded in the source module

_CACHE = {}


def _build_bass():
    import concourse.bass as bass
    import concourse.tile as tile
    from concourse import bacc, mybir

    f32 = mybir.dt.float32
    bf16 = mybir.dt.bfloat16
    nc = bacc.Bacc("TRN2", target_bir_lowering=False, debug=False, num_devices=NCORES)

    # one combined operand tile (host-packed, bf16): cols 0:128 = m1
    # (mm2 rhs), 128:256 = m0 (mm1 rhs), 256:321 = spT, 322:386 = ones
    # rows, 386:387 = runtime g cell
    CC = 388
    c_d = nc.dram_tensor("comb", [128, CC], bf16, kind="ExternalInput")
    n_d = nc.dram_tensor("noise2", [COLS, 128], bf16, kind="ExternalInput")
    i_d = nc.dram_tensor("id64", [COLS, COLS], bf16, kind="ExternalInput")
    o_d = nc.dram_tensor("out", [COLS, 128], f32, kind="ExternalOutput")

    with tile.TileContext(nc) as tc:
        with (
            tc.tile_pool(name="sb", bufs=1) as sb,
            tc.tile_pool(name="ps", bufs=1, space="PSUM") as ps,
        ):
            # ---- input DMAs: spike grid halves split across both HWDGE
            # rings, kernel table on sync, noise on scalar ----
            # ---- input DMAs. Everything bf16 rides one combined tile,
            # row-split across both HWDGE rings (two ~64-row DIRECT2Ds
            # issue in parallel; 192 descriptor-rows total instead of
            # 322), so the conv operands land ~1us earlier. noise2 (f32)
            # follows on sync; the SWDGE ring is unused. ----
            comb = sb.tile([128, CC], bf16, tag="L")
            nrm = sb.tile([COLS, 128], bf16)
            id64 = sb.tile([COLS, COLS], bf16)
            nc.sync.dma_start(
                out=comb[0:64, :],
                in_=bass.AP(tensor=c_d.ap().tensor, offset=0, ap=[[CC, 64], [1, CC]]),
            )
            nc.scalar.dma_start(
                out=comb[64:128, :],
                in_=bass.AP(
                    tensor=c_d.ap().tensor, offset=64 * CC, ap=[[CC, 64], [1, CC]]
                ),
            )
            nc.sync.dma_start(out=nrm[:], in_=n_d.ap())
            nc.scalar.dma_start(out=id64[:], in_=i_d.ap())
            # ---- banded conv: psedu_rm[c, p] = sum_qt spT[qt, c+b] M_b[qt, p] ----
            conv_ps = ps.tile([COLS, 128], f32, tag="ps_c")
            stat_ps = ps.tile([COLS, 2], f32, tag="ps_s")
            nc.tensor.matmul(
                conv_ps[:], lhsT=comb[:, 256:320], rhs=comb[:, 128:256],
                start=True, stop=False,
            )
            nc.tensor.matmul(
                conv_ps[:], lhsT=comb[:, 257:321], rhs=comb[:, 0:128],
                start=False, stop=True,
            )

            # ---- fused noise-add + row-max, then row-min ----
            # ---- noise add, then per-row max / -min (bf16 ps_rm gives no
            # DVE speedup on TRN2 - the 16-bit 2x path doesn't engage - so
            # keep f32 for accuracy; pk is bf16 for the bf16 transpose) ----
            ps_rm = sb.tile([COLS, 128], f32)
            pk = sb.tile([COLS, 2], bf16)  # col0 = -row min, col1 = row max
            nc.vector.tensor_tensor(
                out=ps_rm[:], in0=conv_ps[:], in1=nrm[:], op=mybir.AluOpType.add,
            )
            nc.vector.tensor_reduce(
                out=pk[:, 1:2], in_=ps_rm[:], axis=mybir.AxisListType.X,
                op=mybir.AluOpType.max,
            )
            nc.vector.tensor_reduce(
                out=pk[:, 0:1], in_=ps_rm[:], axis=mybir.AxisListType.X,
                op=mybir.AluOpType.min, negate=True,
            )
            # ---- global min/max: PE transpose of pk (bf16 identity from
            # the combined tile), one column reduce into the g cell, then
            # two ones-row broadcast matmuls ----
            pk_ps = ps.tile([2, COLS], bf16, tag="ps_k")
            nc.tensor.transpose(pk_ps[:], pk[:], id64[:])
            nc.vector.tensor_reduce(
                out=comb[0:2, 386:387], in_=pk_ps[:], axis=mybir.AxisListType.X,
                op=mybir.AluOpType.max,
            )  # [-gmin, gmax]
            # two ones-row broadcast matmuls off the same g cell:
            # contraction over both rows -> range; over row 0 -> -gmin
            nc.tensor.matmul(
                stat_ps[:, 0:1], lhsT=comb[0:2, 322:386], rhs=comb[0:2, 386:387],
                start=True, stop=True,
            )
            nc.tensor.matmul(
                stat_ps[:, 1:2], lhsT=comb[0:1, 322:386], rhs=comb[0:1, 386:387],
                start=True, stop=True,
            )  # stat[:, 0] = range, stat[:, 1] = -gmin
            inv_rng = sb.tile([COLS, 1], f32)
            nc.vector.reciprocal(inv_rng[:], stat_ps[:, 0:1])
            # single fused normalize (x + (-gmin)) * 1/range, then store
            # halves on both HWDGE rings so the issues run in parallel
            outt = sb.tile([COLS, 128], f32)
            nc.vector.tensor_scalar(
                out=outt[:], in0=ps_rm[:], scalar1=stat_ps[:, 1:2],
                scalar2=inv_rng[:, 0:1], op0=mybir.AluOpType.add,
                op1=mybir.AluOpType.mult,
            )
            nc.sync.dma_start(out=o_d.ap()[0:32, :], in_=outt[0:32, :])
            nc.scalar.dma_start(out=o_d.ap()[32:COLS, :], in_=outt[32:COLS, :])

    nc.compile()
    return nc


def _get_nc():
    if "nc" not in _CACHE:
        _CACHE["nc"] = _build_bass()
    return _CACHE["nc"]


def _run(in_map, trace=False, **kwargs):
    from concourse.bass_utils import run_bass_kernel_spmd

    nc = _get_nc()
    return run_bass_kernel_spmd(
        nc, [in_map] * NCORES, core_ids=list(range(NCORES)), trace=trace, **kwargs
    )


def _prepare(X, weight, noise, sigma):
    """Host-side input prep: fold the kernel bank into the banded-Toeplitz
    conv operand (linearity of the softmax-weighted sum; min-max norm is
    invariant to the overall softmax scale, which lands on the noise),
    threshold + shift-transpose the spike train."""
    import ml_dtypes

    X = np.ascontiguousarray(X, dtype=np.float32).reshape(T)
    weight = np.asarray(weight, dtype=np.float64).reshape(D)
    noise = np.ascontiguousarray(noise, dtype=np.float32).reshape(COLS, 128)
    sigma = np.asarray(sigma, dtype=np.float64).reshape(D)

    # kbar'(delta) = sum_d exp(w_d)/sigma_d * exp(-delta^2/(2 sigma_d^2));
    # psedu' = kbar' (*) spikes + (esum/C)*noise is a positive affine image
    # of the reference psedu, so the min-max normalized output matches.
    delta = np.arange(-256, 256, dtype=np.float64)
    kb = (
        (np.exp(weight) / sigma)[:, None]
        * np.exp(-(delta[None, :] ** 2) / (2.0 * sigma * sigma)[:, None])
    ).sum(0)  # kb[j] = kbar'(j - 256)

    # m_all[qt, p + 128*(1-b)] = kbar'(p + 44 - qt - 128b), b in {0, 1}
    qt = np.arange(128)[:, None]
    p = np.arange(128)[None, :]
    m0 = kb[256 + p + 44 - qt]  # b=0 -> comb cols 128:256
    m1 = kb[256 + p + 44 - qt - 128]  # b=1 -> comb cols 0:128

    # spT[qt, c'] = spikes[128c' - 45 + qt], zero outside [0, T)
    spikes = (X > 0.5).astype(np.float32)
    c = np.arange(SCOLS)[None, :]
    idx = 128 * c - SHIFT + np.arange(128)[:, None]
    valid = (idx >= 0) & (idx < T)
    spT = np.where(valid, spikes[np.clip(idx, 0, T - 1)], 0.0)

    es = np.exp(weight).sum() / GAUSS_C
    noise2 = (es * noise.astype(np.float64)).astype(np.float32)

    # combined bf16 tile: [m1 | m0 | spT | pad | ones rows | g cell],
    # matching the column offsets hardcoded on-device
    comb = np.zeros((128, 388), dtype=np.float64)
    comb[:, 0:128] = m1
    comb[:, 128:256] = m0
    comb[:, 256 : 256 + SCOLS] = spT
    comb[0:2, 322:386] = 1.0  # ones rows for the broadcast matmuls

    return {
        "comb": comb.astype(ml_dtypes.bfloat16),
        "noise2": noise2.astype(ml_dtypes.bfloat16),
        "id64": np.eye(COLS, dtype=np.float64).astype(ml_dtypes.bfloat16),
    }


def kernel(X, weight, noise, sigma):
    in_map = _prepare(X, weight, noise, sigma)
    try:
        res = _run(in_map).results
    except Exception:
        # transient runtime INTERNAL errors (device wedge) clear on retry
        res = _run(in_map).results
    return res[0]["out"].reshape(1, T)
